# revision 20
# baseline (speedup 1.0000x reference)
"""Causal self-attention (B=4, T=2048, C=1024, H=16) on 8 TRN2 NeuronCores.

Sharding: core = (batch, head_group): 4 batches x 2 groups of 8 heads.
Each core computes, for its batch b and head group g:
  - q^T/k^T slices (features for its 8 heads, transposed layout [feat, tok])
  - v in natural layout [tok, feat] via x-stationary matmuls (no PE transposes)
  - causal attention for its 8 heads (scores^T tiles in PSUM, exp on ACT,
    fused softmax-denominator via a ones-column in the AV matmul)
  - its 512-row slice of the output projection (row-parallel c_proj)
Host sums the two per-batch partials and adds b_proj (the "all-reduce").

Engine assignment: PE = matmuls only; ACT = exp only; DVE = evictions, masks,
norm.  The PE stream interleaves qkv/c_proj "fill" matmuls between attention
steps so the PE never starves while ACT works through the exps.  Tile's
hazard tracking is coarse (tile-granular, not range-granular), so every
tensor that is written in one phase and read in another is split into
per-phase tiles (qkT by token range, vaug by k-tile range, ctx by q-chunk)
-- otherwise fill-eviction writes serialize against attention reads.
Softmax normalization chains (denominator reshape -> reciprocal ->
broadcast, two DRAM round-trips) are software-pipelined one chunk behind
the attention; for the last chunk the chain is staggered per head-pair and
covered by held-back c_proj work.
"""

import numpy as np
import ml_dtypes

B, T, C, H, D = 4, 2048, 1024, 16, 64
NC_ = 8            # cores
HPC = 8            # heads per core
GF = 512           # features per head-group (8 heads * 64)
NT = T // 128      # 16 token tiles
NQC = T // 512     # 4 q-chunks
PW = 192           # per-head-PAIR stride in vaug: [v_h0 | ones64 | v_h1]
BF16 = ml_dtypes.bfloat16

_nc_cache = {}


def _build(with_bias=False):
    import concourse.bacc as bacc
    import concourse.tile as tile
    import concourse.mybir as mybir
    import concourse.bass as bass

    mbf = mybir.dt.bfloat16
    mf32 = mybir.dt.float32
    ACT = mybir.ActivationFunctionType

    nc = bacc.Bacc("TRN2", target_bir_lowering=False)
    xT_d = nc.dram_tensor("xT", [4, 128, 4096], mbf, kind="ExternalInput")
    wqk_d = nc.dram_tensor("wqk", [8, 128, 1024], mbf, kind="ExternalInput")
    wvT_d = nc.dram_tensor("wvT", [128, 8, 512], mbf, kind="ExternalInput")
    bias_d = nc.dram_tensor("bias", [128, 8], mf32, kind="ExternalInput")
    bv_d = nc.dram_tensor("bv", [1, 512], mf32, kind="ExternalInput")
    wp_d = nc.dram_tensor("wp", [128, 4096], mbf, kind="ExternalInput")
    cmask_d = nc.dram_tensor("cmask", [128, 256], mbf, kind="ExternalInput")
    out_d = nc.dram_tensor("out", [T, C], mbf, kind="ExternalOutput")
    rU_d = nc.dram_tensor("rU_scratch", [32, 512], mbf, kind="Internal")
    sD_d = nc.dram_tensor("sD_scratch", [64, 256], mbf, kind="Internal")

    with tile.TileContext(nc) as tc:
        with tc.tile_pool(name="const", bufs=1) as cpool, \
             tc.tile_pool(name="big", bufs=1) as big, \
             tc.tile_pool(name="pp", bufs=8) as ppool, \
             tc.tile_pool(name="rbp", bufs=8) as rbpool, \
             tc.tile_pool(name="st", bufs=4) as stpool, \
             tc.tile_pool(name="outp", bufs=6) as outpool, \
             tc.tile_pool(name="ps_a", bufs=2, space="PSUM") as ps_a, \
             tc.tile_pool(name="ps_sc", bufs=2, space="PSUM") as ps_sc, \
             tc.tile_pool(name="ps_ctx", bufs=2, space="PSUM") as ps_ctx:

            # ---- inputs to SBUF, ordered by first use ----
            xT = big.tile([128, 4, 8, 512], mbf, tag="xT")
            wqk = big.tile([128, 8, 8, 128], mbf, tag="wqk")
            wvT = big.tile([128, 8, 512], mbf, tag="wvT")
            # weights go out on the ACT HWDGE ring, x on the SP ring --
            # the two rings load in parallel so the first window's inputs
            # arrive in ~max() rather than sum() of the transfer times
            nc.scalar.dma_start(
                out=wqk[:, 0, :, :],
                in_=wqk_d[0, :, :].rearrange("p (e c) -> p e c", e=8))
            nc.scalar.dma_start(
                out=wqk[:, 4, :, :],
                in_=wqk_d[4, :, :].rearrange("p (e c) -> p e c", e=8))
            nc.sync.dma_start(
                out=xT[:, 0, :, :],
                in_=xT_d[0, :, :].rearrange("p (e t) -> p e t", e=8))
            nc.sync.dma_start(
                out=xT[:, 1, :, :],
                in_=xT_d[1, :, :].rearrange("p (e t) -> p e t", e=8))
            cmask = cpool.tile([128, 256], mbf, tag="cmask")
            nc.scalar.dma_start(out=cmask, in_=cmask_d[:, :])
            nc.scalar.dma_start(out=wvT, in_=wvT_d[:, :, :])
            if with_bias:
                bias = cpool.tile([128, 8], mf32, tag="bias")
                nc.sync.dma_start(out=bias, in_=bias_d[:, :])
                bvb = cpool.tile([128, 512], mf32, tag="bvb")
                base = bv_d[0:1, :]
                bcast = bass.AP(tensor=base.tensor, offset=base.offset,
                                ap=[[0, 128], [1, 512]])
                nc.sync.dma_start(out=bvb, in_=bcast)
            for f in (1, 5, 2, 6, 3, 7):
                nc.scalar.dma_start(
                    out=wqk[:, f, :, :],
                    in_=wqk_d[f, :, :].rearrange("p (e c) -> p e c", e=8))
            nc.sync.dma_start(
                out=xT[:, 2, :, :],
                in_=xT_d[2, :, :].rearrange("p (e t) -> p e t", e=8))
            nc.sync.dma_start(
                out=xT[:, 3, :, :],
                in_=xT_d[3, :, :].rearrange("p (e t) -> p e t", e=8))
            wp = cpool.tile([128, 4, 1024], mbf, tag="wp")
            nc.scalar.dma_start(
                out=wp, in_=wp_d[:, :].rearrange("p (e t) -> p e t", e=4))

            # persistent intermediates.  Tile's dependency tracking is
            # coarse, so tensors written in one phase and read in a later
            # one are SPLIT into per-phase tiles to avoid false hazards:
            #   qkT:  a = token cols 0:1024 (built up front),
            #         b = 1024:1536 (fill in c1), c = 1536:2048 (fill c2)
            #   vaug: a = k-tiles 0-7, b = 8-11, c = 12-15
            #   ctx:  one tile per q-chunk
            qkTa = big.tile([128, 8, 1024], mbf, tag="qkTa")
            qkTb = big.tile([128, 8, 512], mbf, tag="qkTb")
            qkTc = big.tile([128, 8, 512], mbf, tag="qkTc")
            vauga = big.tile([128, 8, 4 * PW], mbf, tag="vauga")
            vaugb = big.tile([128, 4, 4 * PW], mbf, tag="vaugb")
            vaugc = big.tile([128, 4, 4 * PW], mbf, tag="vaugc")
            ctx4 = [big.tile([128, 4, 512], mbf, tag=f"ctx{c}",
                             name=f"ctx{c}")
                    for c in range(4)]
            sS = big.tile([64, 4, 64], mbf, tag="sS")        # softmax denoms
            rU = big.tile([64, 4, 64], mbf, tag="rU")

            def qk_ap(f, qc, rows=slice(None)):
                """qkT slice for feature chunk f, 512-token chunk qc."""
                if qc < 2:
                    return qkTa[rows, f, qc * 512:(qc + 1) * 512]
                return (qkTb if qc == 2 else qkTc)[rows, f, :]

            def k_ap(rows, f, kt):
                """k^T slice [64, 128] for one 128-token k-tile."""
                if kt < 8:
                    return qkTa[rows, f, kt * 128:(kt + 1) * 128]
                if kt < 12:
                    return qkTb[rows, f, (kt - 8) * 128:(kt - 7) * 128]
                return qkTc[rows, f, (kt - 12) * 128:(kt - 11) * 128]

            def v_tile(kt):
                if kt < 8:
                    return vauga, kt
                if kt < 12:
                    return vaugb, kt - 8
                return vaugc, kt - 12

            # HAM warm-up: keep the PE busy through the initial input-DMA
            # wait so the first real matmuls run at 2.4 GHz.
            warm = cpool.tile([128, 128], mbf, tag="warm")
            nc.vector.memset(warm, 0.0)
            wps = ps_sc.tile([128, 128], mf32, tag="sc", name="warmps")
            for i in range(60):
                nc.tensor.matmul(wps, warm, warm, start=(i == 0),
                                 stop=(i == 59))
            # load the exp table while the PE warms (first ACTIVATE pays
            # ~2.7us of table DMA otherwise)
            wexp = cpool.tile([128, 128], mbf, tag="wexp")
            nc.scalar.activation(wexp, wps, ACT.Exp, scale=0.125)

            # shared ones block of vaug: per pair [v_h0 | ones | v_h1],
            # so h0's stationary [v|ones] puts ctx on psum rows 0:64 and
            # h1's [ones|v] puts ctx on rows 64:128 -- both lane-aligned
            # with their ctx destination (no cross-partition bounce), and
            # rows 64:128 / 0:64 carry the softmax denominator replicas.
            for va in (vauga, vaugb, vaugc):
                vv = va.rearrange("p t (pr w) -> p t pr w", w=PW)
                nc.vector.memset(vv[:, :, :, 64:128], 1.0)

            def qk_evict(dst, acc, f):
                if with_bias:
                    nc.vector.tensor_scalar_add(dst, acc, bias[:, f:f + 1])
                else:
                    nc.vector.tensor_copy(dst, acc)

            def gen_qk_pair(fa, fb, qc):
                """One 512-col window of qkT for two f-chunks, matmuls
                interleaved (alternating psum banks), evictions on DVE."""
                acca = ps_a.tile([128, 512], mf32, tag="qkvp",
                                 name=f"qkq_{fa}_{qc}")
                accb = ps_a.tile([128, 512], mf32, tag="qkvp",
                                 name=f"qkq_{fb}_{qc}")
                for e in range(8):
                    nc.tensor.matmul(acca, wqk[:, fa, e, :],
                                     xT[:, qc, e, :],
                                     start=(e == 0), stop=(e == 7))
                    yield
                    nc.tensor.matmul(accb, wqk[:, fb, e, :],
                                     xT[:, qc, e, :],
                                     start=(e == 0), stop=(e == 7))
                    yield
                qk_evict(qk_ap(fa, qc), acca, fa)
                qk_evict(qk_ap(fb, qc), accb, fb)

            def vnat_evict(t, acc):
                va, ti = v_tile(t)
                vv = va.rearrange("p t (pr w) -> p t pr w", w=PW)
                src = acc.rearrange("p (pr j d) -> p pr j d", pr=4, j=2)
                bb = (bvb.rearrange("p (pr j d) -> p pr j d", pr=4, j=2)
                      if with_bias else None)
                for j, cols in ((0, slice(0, 64)), (1, slice(128, 192))):
                    if with_bias:
                        nc.vector.tensor_add(
                            vv[:, ti, :, cols], src[:, :, j, :], bb[:, :, j, :])
                    else:
                        nc.vector.tensor_copy(
                            vv[:, ti, :, cols], src[:, :, j, :])

            def gen_vnat_pair(ta, tb):
                """v rows (tokens) for two 128-token tiles, natural layout,
                x-stationary: v[t,:] = x[t,:] @ w_v."""
                acca = ps_a.tile([128, 512], mf32, tag="qkvp",
                                 name=f"vna_{ta}")
                accb = ps_a.tile([128, 512], mf32, tag="qkvp",
                                 name=f"vnb_{tb}")
                for e in range(8):
                    nc.tensor.matmul(
                        acca, xT[:, ta // 4, e, (ta % 4) * 128:(ta % 4 + 1) * 128],
                        wvT[:, e, :], start=(e == 0), stop=(e == 7))
                    yield
                    nc.tensor.matmul(
                        accb, xT[:, tb // 4, e, (tb % 4) * 128:(tb % 4 + 1) * 128],
                        wvT[:, e, :], start=(e == 0), stop=(e == 7))
                    yield
                vnat_evict(ta, acca)
                vnat_evict(tb, accb)

            def gen_cproj_t(t):
                """out[t-block] = ctx @ wp (row-parallel slice, f32), both
                512-col halves interleaved."""
                ct = ctx4[t // 4]
                tt = t % 4
                osb = outpool.tile([128, 1024], mbf, tag="osb",
                                   name=f"osb_{t}")
                pa = ps_a.tile([128, 512], mf32, tag="qkvp", name=f"cpa_{t}")
                pb = ps_a.tile([128, 512], mf32, tag="qkvp", name=f"cpb_{t}")
                for fc in range(4):
                    nc.tensor.matmul(pa, ct[:, fc, tt * 128:(tt + 1) * 128],
                                     wp[:, fc, 0:512],
                                     start=(fc == 0), stop=(fc == 3))
                    yield
                    nc.tensor.matmul(pb, ct[:, fc, tt * 128:(tt + 1) * 128],
                                     wp[:, fc, 512:1024],
                                     start=(fc == 0), stop=(fc == 3))
                    yield
                nc.vector.tensor_copy(osb[:, 0:512], pa)
                nc.vector.tensor_copy(osb[:, 512:1024], pb)
                nc.sync.dma_start(out=out_d[t * 128:(t + 1) * 128, :], in_=osb)

            class FillQueue:
                def __init__(self):
                    self.gens = []
                    self.cur = None

                def add(self, g):
                    self.gens.append(g)

                def pull(self, n):
                    for _ in range(n):
                        while True:
                            if self.cur is None:
                                if not self.gens:
                                    return
                                self.cur = self.gens.pop(0)
                            try:
                                next(self.cur)
                                break
                            except StopIteration:
                                self.cur = None

                def drain(self):
                    self.pull(1 << 30)

            fq = FillQueue()

            def attention_chunk(g2, c, nfill):
                nkt = 4 * c + 4
                ctxp = [ps_ctx.tile([128, 512], mf32, tag="ctx",
                                    name=f"ctxp{g2}_{c}_{jj}")
                        for jj in range(2)]

                def emit_ctx(kt, pv, off):
                    va, ti = v_tile(kt)
                    for j in range(2):
                        base = g2 * PW + 64 * j
                        nc.tensor.matmul(
                            ctxp[j][:, off:],
                            va[:, ti, base:base + 128],
                            pv[:, j, off:],
                            start=(kt == 0), stop=(kt == nkt - 1))

                pending_ctx = None
                for kt in range(nkt):
                    # Both heads' score matmuls row-tiled (concurrent in the
                    # PE array); halves of one [128,1024] psum tile ->
                    # single merged exp.  Diagonal k-tiles (m>=0) use exact
                    # column ranges.  The A.V matmul for kt is emitted after
                    # the scores of kt+1 so the exp it consumes has a k-tile
                    # of pipeline slack; fill matmuls sit before the A.V so
                    # the PE works while ACT does exp.
                    m = kt - 4 * c
                    off = 128 * m if m > 0 else 0
                    sc = ps_sc.tile([128, 1024], mf32, tag="sc",
                                    name=f"sc_{g2}_{c}_{kt}")
                    scv = sc.rearrange("r (j q) -> r j q", j=2)
                    for j in range(2):
                        rows = slice(64 * j, 64 * (j + 1))
                        nc.tensor.matmul(
                            scv[:, j, off:],
                            k_ap(rows, 4 + g2, kt),
                            qk_ap(g2, c, rows)[:, off:],
                            start=True, stop=True,
                            tile_position=(64 * j, 0))
                    p = ppool.tile([128, 1024], mbf, tag="p")
                    pv = p.rearrange("r (j q) -> r j q", j=2)
                    nc.scalar.activation(pv[:, :, off:], scv[:, :, off:],
                                         ACT.Exp, scale=0.125)
                    if m >= 0:
                        # lower-tri mask on the 128-wide diagonal block
                        nc.vector.tensor_mul(
                            pv[:, :, off:off + 128],
                            pv[:, :, off:off + 128],
                            cmask.rearrange("r (j q) -> r j q", j=2))
                    fq.pull(nfill)
                    if pending_ctx is not None:
                        emit_ctx(*pending_ctx)
                    pending_ctx = (kt, pv, off)
                emit_ctx(*pending_ctx)
                # the AV output is lane-aligned with its ctx destination
                # (j=0 ctx on psum rows 0:64, j=1 on 64:128); rows 64:128 /
                # 0:64 hold the denominator replicas.  Denominator rows go
                # to DRAM scratch reshaped [8 rows, 64] so the reciprocal
                # runs on 64 lanes with a 64-wide free dim.
                for j in range(2):
                    h = 2 * g2 + j
                    st = stpool.tile([65, 512], mbf, tag="st65",
                                     name=f"st_{g2}_{c}_{j}")
                    srow = 64 if j == 0 else 0
                    nc.vector.tensor_copy(st[srow:srow + 1, :],
                                          ctxp[j][srow:srow + 1, :])
                    nc.sync.dma_start(
                        out=sD_d[8 * h:8 * h + 8, 64 * c:64 * (c + 1)],
                        in_=st[srow:srow + 1, :])
                    nc.vector.tensor_copy(
                        ctx4[c][64 * j:64 * (j + 1), g2, :],
                        ctxp[j][64 * j:64 * (j + 1), :])

            # --- softmax-normalization chain, split so no engine FIFO ever
            # waits on a DMA round-trip: gather (DMA) emitted right after a
            # chunk; reciprocal + broadcast emitted ~a chunk later. ---
            def norm_gather(c, g2s=(0, 4)):
                lo, hi = 16 * g2s[0], 16 * g2s[1]
                nc.sync.dma_start(out=sS[lo:hi, c, :],
                                  in_=sD_d[lo:hi, 64 * c:64 * (c + 1)])

            def norm_finish(c, g2s=(0, 1, 2, 3), recip=True):
                ng = len(g2s)
                if recip:
                    lo, hi = 16 * g2s[0], 16 * (g2s[-1] + 1)
                    with nc.allow_low_precision(reason="1/s bf16 is plenty"):
                        nc.vector.reciprocal(rU[lo:hi, c, :], sS[lo:hi, c, :])
                    r0 = 8 * c + 2 * g2s[0]
                    r1 = 8 * c + 2 * g2s[-1] + 2
                    nc.sync.dma_start(
                        out=rU_d[r0:r1, :].rearrange(
                            "h (r q) -> (h r) q", r=8),
                        in_=rU[lo:hi, c, :])
                rb = rbpool.tile([128, 4, 512], mbf, tag="rb",
                                 name=f"rb_{c}_{g2s[0]}")
                for j in range(2):
                    base = rU_d[8 * c + 2 * g2s[0] + j:
                                8 * c + 2 * g2s[0] + j + 1, :]
                    bcast = bass.AP(tensor=base.tensor, offset=base.offset,
                                    ap=[[0, 64], [1024, ng], [1, 512]])
                    nc.sync.dma_start(out=rb[64 * j:64 * (j + 1), 0:ng, :],
                                      in_=bcast)
                return [(rb, i) for i in range(ng)]

            def norm_mul(c, rbs, g2s=(0, 1, 2, 3)):
                """ctx[c] *= 1/s (in place, bf16 2x mode)."""
                for (rb, i), g2 in zip(rbs, g2s):
                    for j in range(2):
                        sl = ctx4[c][64 * j:64 * (j + 1), g2, :]
                        nc.vector.tensor_mul(
                            sl, sl, rb[64 * j:64 * (j + 1), i, :])

            # ---- emission order = per-engine execution order ----
            # P1: q,k windows + v tiles, attention c0 staggered one window
            # pair behind its dependencies.
            for g in [gen_qk_pair(0, 4, 0), gen_qk_pair(0, 4, 1),
                      gen_vnat_pair(0, 1), gen_vnat_pair(2, 3),
                      gen_qk_pair(1, 5, 0), gen_qk_pair(1, 5, 1)]:
                for _ in g:
                    pass
            attention_chunk(0, 0, 0)
            for g in [gen_qk_pair(2, 6, 0), gen_qk_pair(2, 6, 1)]:
                for _ in g:
                    pass
            attention_chunk(1, 0, 0)
            for g in [gen_qk_pair(3, 7, 0), gen_qk_pair(3, 7, 1)]:
                for _ in g:
                    pass
            fq.add(gen_vnat_pair(4, 5))
            attention_chunk(2, 0, 2)
            fq.add(gen_vnat_pair(6, 7))
            attention_chunk(3, 0, 3)
            fq.drain()
            norm_gather(0)

            # c1: fill = qc=2 q,k windows; finish(0) after first chunk
            for fa, fb in [(0, 4), (1, 5), (2, 6), (3, 7)]:
                fq.add(gen_qk_pair(fa, fb, 2))
            attention_chunk(0, 1, 2)
            rbs0 = norm_finish(0)
            for g2 in range(1, 4):
                attention_chunk(g2, 1, 2)
            fq.drain()              # qc=2 must be done before c2 scores
            norm_gather(1)

            # c2: v tiles 8-11 precede (AV deps); fill = three qc=3
            # windows, v 12-15, then c0's first c_proj tiles
            for g in [gen_vnat_pair(8, 9), gen_vnat_pair(10, 11)]:
                for _ in g:
                    pass
            for fa, fb in [(0, 4), (1, 5), (2, 6)]:
                fq.add(gen_qk_pair(fa, fb, 3))
            fq.add(gen_vnat_pair(12, 13))
            fq.add(gen_vnat_pair(14, 15))
            attention_chunk(0, 2, 2)
            rbs1 = norm_finish(1)
            norm_mul(0, rbs0)
            for t in (0, 1):
                fq.add(gen_cproj_t(t))
            for g2 in range(1, 4):
                attention_chunk(g2, 2, 2)
            fq.drain()              # qc=3 (0-2) + v 12-15 before c3
            norm_gather(2)

            # c3: fill = last qc=3 window + c0/c1 c_proj; c2's c_proj is
            # held back to cover the tail.  The c3 normalization chain is
            # staggered per head-pair, one chunk behind its denominators.
            norm_mul(1, rbs1)
            fq.add(gen_qk_pair(3, 7, 3))
            for t in (2, 3):
                fq.add(gen_cproj_t(t))
            for t in range(4, 8):
                fq.add(gen_cproj_t(t))
            attention_chunk(0, 3, 2)
            rbs2 = norm_finish(2)
            norm_gather(3, (0, 1))
            attention_chunk(1, 3, 2)
            norm_mul(2, rbs2)
            norm_gather(3, (1, 2))
            attention_chunk(2, 3, 2)
            rb3 = norm_finish(3, (0, 1))
            norm_gather(3, (2, 3))
            attention_chunk(3, 3, 2)
            fq.drain()
            norm_gather(3, (3, 4))
            # tail: held-back c_proj (reads ctx[2], independent of the c3
            # norm chain) covers the g2=3 normalization round-trips; paced
            # dummy matmuls keyed on the chain keep HAM from re-throttling
            for t in (8, 9, 10, 11):
                fq.add(gen_cproj_t(t))
            fq.pull(32)
            wd1 = ps_sc.tile([128, 512], mf32, tag="sc", name="wd1")
            for i in range(4):
                nc.tensor.matmul(wd1[:, 0:64], warm[0:64, :], sS[:, 3, :],
                                 start=(i == 0), stop=(i == 3))
            rb3 += norm_finish(3, (2, 3))
            fq.drain()
            wd15 = ps_sc.tile([128, 64], mf32, tag="sc", name="wd15")
            for i in range(4):
                nc.tensor.matmul(wd15, warm[0:64, :], rU[:, 3, :],
                                 start=(i == 0), stop=(i == 3))
            wd2 = ps_sc.tile([128, 512], mf32, tag="sc", name="wd2")
            for i in range(3):
                nc.tensor.matmul(wd2, warm, rb3[2][0][:, 0, :],
                                 start=(i == 0), stop=(i == 2))
            norm_mul(3, rb3[0:1], (0,))
            norm_mul(3, rb3[1:2], (1,))
            norm_mul(3, rb3[2:3], (2,))
            norm_mul(3, rb3[3:4], (3,))
            for t in range(12, 16):
                for _ in gen_cproj_t(t):
                    pass

    nc.compile()
    return nc


def _prep_inputs(x, w_attn, b_attn, w_proj):
    """Host-side shard/layout prep for the 8 cores."""
    # causal masks: cmask[:, j*128 + q] = 1.0 iff q >= k_row
    k_r = np.arange(128)[:, None]
    q_i = np.arange(128)[None, :]
    tri = (q_i >= k_r)
    cmask = np.concatenate([tri, tri], axis=1).astype(BF16)  # [128, 256]

    # xT[qc, p, e*512+t'] = x[qc*512+t', e*128+p]: per-partition rows
    # are contiguous so input DMAs run at full bandwidth
    xT_b = [np.ascontiguousarray(
        x[b].T.reshape(8, 128, 4, 512).transpose(2, 1, 0, 3).reshape(
            4, 128, 4096)).astype(BF16) for b in range(B)]
    in_maps = []
    for core in range(NC_):
        b, g = core // 2, core % 2
        fsl = slice(g * GF, (g + 1) * GF)
        # q,k: [8, 128, 1024]; wqk[f, p, e*128+c] = w[e*128+p, base_f+c]
        wq = w_attn[:, fsl].reshape(8, 128, 4, 128)
        wk = w_attn[:, C + g * GF:C + (g + 1) * GF].reshape(8, 128, 4, 128)
        wqk = np.concatenate(
            [wq.transpose(2, 1, 0, 3).reshape(4, 128, 1024),
             wk.transpose(2, 1, 0, 3).reshape(4, 128, 1024)],
            axis=0).astype(BF16)
        # v: [128, 8, 512]; wvT[p, e, c] = w[e*128+p, 2C+g*512+c]
        wv = w_attn[:, 2 * C + g * GF:2 * C + (g + 1) * GF]
        wvT = np.ascontiguousarray(
            wv.reshape(8, 128, 512).transpose(1, 0, 2)).astype(BF16)
        bq = b_attn[fsl]
        bk = b_attn[C + g * GF:C + (g + 1) * GF]
        bias = np.stack(
            [np.concatenate([bq, bk])[f * 128:(f + 1) * 128]
             for f in range(8)], axis=1).astype(np.float32)
        bv = b_attn[2 * C + g * GF:2 * C + (g + 1) * GF].reshape(1, 512)
        bv = np.ascontiguousarray(bv).astype(np.float32)
        # wp[p, e*1024+t] = w_proj[g*512+e*128+p, t] (contiguous rows)
        wp = np.ascontiguousarray(
            w_proj[fsl, :].reshape(4, 128, 1024).transpose(1, 0, 2).reshape(
                128, 4096)).astype(BF16)
        in_maps.append({"xT": xT_b[b], "wqk": wqk, "wvT": wvT, "bias": bias,
                        "bv": bv, "wp": wp, "cmask": cmask})
    return in_maps


def _run(in_maps, trace=False, with_bias=False):
    from concourse.bass_utils import run_bass_kernel_spmd
    if with_bias not in _nc_cache:
        _nc_cache[with_bias] = _build(with_bias)
    return run_bass_kernel_spmd(_nc_cache[with_bias], in_maps,
                                core_ids=list(range(NC_)), trace=trace)


def kernel(x, w_attn, b_attn, w_proj, b_proj):
    x = np.asarray(x, dtype=np.float32)
    w_attn = np.asarray(w_attn, dtype=np.float32)
    b_attn = np.asarray(b_attn, dtype=np.float32)
    w_proj = np.asarray(w_proj, dtype=np.float32)
    b_proj = np.asarray(b_proj, dtype=np.float32)
    res = _run(_prep_inputs(x, w_attn, b_attn, w_proj),
               with_bias=bool(np.any(b_attn)))
    out = np.empty((B, T, C), np.float32)
    for b in range(B):
        out[b] = (res.results[2 * b]["out"].astype(np.float32)
                  + res.results[2 * b + 1]["out"].astype(np.float32)
                  + b_proj)
    return out


# revision 21
# speedup vs baseline: 1.0066x; 1.0066x over previous
"""Causal self-attention (B=4, T=2048, C=1024, H=16) on 8 TRN2 NeuronCores.

Sharding: core = (batch, head_group): 4 batches x 2 groups of 8 heads.
Each core computes, for its batch b and head group g:
  - q^T/k^T slices (features for its 8 heads, transposed layout [feat, tok])
  - v in natural layout [tok, feat] via x-stationary matmuls (no PE transposes)
  - causal attention for its 8 heads (scores^T tiles in PSUM, exp on ACT,
    fused softmax-denominator via a ones-column in the AV matmul)
  - its 512-row slice of the output projection (row-parallel c_proj)
Host sums the two per-batch partials and adds b_proj (the "all-reduce").

Engine assignment: PE = matmuls only; ACT = exp only; DVE = evictions, masks,
norm.  The PE stream interleaves qkv/c_proj "fill" matmuls between attention
steps so the PE never starves while ACT works through the exps.  Tile's
hazard tracking is coarse (tile-granular, not range-granular), so every
tensor that is written in one phase and read in another is split into
per-phase tiles (qkT by token range, vaug by k-tile range, ctx by q-chunk)
-- otherwise fill-eviction writes serialize against attention reads.
Softmax normalization chains (denominator reshape -> reciprocal ->
broadcast, two DRAM round-trips) are software-pipelined one chunk behind
the attention; for the last chunk the chain is staggered per head-pair and
covered by held-back c_proj work.
"""

import numpy as np
import ml_dtypes

B, T, C, H, D = 4, 2048, 1024, 16, 64
NC_ = 8            # cores
HPC = 8            # heads per core
GF = 512           # features per head-group (8 heads * 64)
NT = T // 128      # 16 token tiles
NQC = T // 512     # 4 q-chunks
PW = 192           # per-head-PAIR stride in vaug: [v_h0 | ones64 | v_h1]
BF16 = ml_dtypes.bfloat16

_nc_cache = {}


def _build(with_bias=False):
    import concourse.bacc as bacc
    import concourse.tile as tile
    import concourse.mybir as mybir
    import concourse.bass as bass

    mbf = mybir.dt.bfloat16
    mf32 = mybir.dt.float32
    ACT = mybir.ActivationFunctionType

    nc = bacc.Bacc("TRN2", target_bir_lowering=False)
    xT_d = nc.dram_tensor("xT", [4, 128, 4096], mbf, kind="ExternalInput")
    wqk_d = nc.dram_tensor("wqk", [8, 128, 1024], mbf, kind="ExternalInput")
    wvT_d = nc.dram_tensor("wvT", [128, 8, 512], mbf, kind="ExternalInput")
    bias_d = nc.dram_tensor("bias", [128, 8], mf32, kind="ExternalInput")
    bv_d = nc.dram_tensor("bv", [1, 512], mf32, kind="ExternalInput")
    wp_d = nc.dram_tensor("wp", [128, 4096], mbf, kind="ExternalInput")
    cmask_d = nc.dram_tensor("cmask", [128, 256], mbf, kind="ExternalInput")
    out_d = nc.dram_tensor("out", [T, C], mbf, kind="ExternalOutput")
    rU_d = nc.dram_tensor("rU_scratch", [32, 512], mbf, kind="Internal")
    sD_d = nc.dram_tensor("sD_scratch", [64, 256], mbf, kind="Internal")

    with tile.TileContext(nc) as tc:
        with tc.tile_pool(name="const", bufs=1) as cpool, \
             tc.tile_pool(name="big", bufs=1) as big, \
             tc.tile_pool(name="pp", bufs=8) as ppool, \
             tc.tile_pool(name="rbp", bufs=8) as rbpool, \
             tc.tile_pool(name="st", bufs=4) as stpool, \
             tc.tile_pool(name="outp", bufs=6) as outpool, \
             tc.tile_pool(name="ps_a", bufs=2, space="PSUM") as ps_a, \
             tc.tile_pool(name="ps_sc", bufs=2, space="PSUM") as ps_sc, \
             tc.tile_pool(name="ps_ctx", bufs=2, space="PSUM") as ps_ctx:

            # ---- inputs to SBUF, ordered by first use ----
            xT = big.tile([128, 4, 8, 512], mbf, tag="xT")
            wqk = big.tile([128, 8, 8, 128], mbf, tag="wqk")
            wvT = big.tile([128, 8, 512], mbf, tag="wvT")
            # weights go out on the ACT HWDGE ring, x on the SP ring --
            # the two rings load in parallel so the first window's inputs
            # arrive in ~max() rather than sum() of the transfer times
            nc.scalar.dma_start(
                out=wqk[:, 0, :, :],
                in_=wqk_d[0, :, :].rearrange("p (e c) -> p e c", e=8))
            nc.scalar.dma_start(
                out=wqk[:, 4, :, :],
                in_=wqk_d[4, :, :].rearrange("p (e c) -> p e c", e=8))
            nc.sync.dma_start(
                out=xT[:, 0, :, :],
                in_=xT_d[0, :, :].rearrange("p (e t) -> p e t", e=8))
            nc.sync.dma_start(
                out=xT[:, 1, :, :],
                in_=xT_d[1, :, :].rearrange("p (e t) -> p e t", e=8))
            cmask = cpool.tile([128, 256], mbf, tag="cmask")
            nc.scalar.dma_start(out=cmask, in_=cmask_d[:, :])
            nc.scalar.dma_start(out=wvT, in_=wvT_d[:, :, :])
            if with_bias:
                bias = cpool.tile([128, 8], mf32, tag="bias")
                nc.sync.dma_start(out=bias, in_=bias_d[:, :])
                bvb = cpool.tile([128, 512], mf32, tag="bvb")
                base = bv_d[0:1, :]
                bcast = bass.AP(tensor=base.tensor, offset=base.offset,
                                ap=[[0, 128], [1, 512]])
                nc.sync.dma_start(out=bvb, in_=bcast)
            for f in (1, 5, 2, 6, 3, 7):
                nc.scalar.dma_start(
                    out=wqk[:, f, :, :],
                    in_=wqk_d[f, :, :].rearrange("p (e c) -> p e c", e=8))
            nc.sync.dma_start(
                out=xT[:, 2, :, :],
                in_=xT_d[2, :, :].rearrange("p (e t) -> p e t", e=8))
            nc.sync.dma_start(
                out=xT[:, 3, :, :],
                in_=xT_d[3, :, :].rearrange("p (e t) -> p e t", e=8))
            wp = cpool.tile([128, 4, 1024], mbf, tag="wp")
            nc.scalar.dma_start(
                out=wp, in_=wp_d[:, :].rearrange("p (e t) -> p e t", e=4))

            # persistent intermediates.  Tile's dependency tracking is
            # coarse, so tensors written in one phase and read in a later
            # one are SPLIT into per-phase tiles to avoid false hazards:
            #   qkT:  a = token cols 0:1024 (built up front),
            #         b = 1024:1536 (fill in c1), c = 1536:2048 (fill c2)
            #   vaug: a = k-tiles 0-7, b = 8-11, c = 12-15
            #   ctx:  one tile per q-chunk
            qkTa = big.tile([128, 8, 1024], mbf, tag="qkTa")
            qkTb = big.tile([128, 8, 512], mbf, tag="qkTb")
            qkTc = big.tile([128, 8, 512], mbf, tag="qkTc")
            vauga = big.tile([128, 8, 4 * PW], mbf, tag="vauga")
            vaugb = big.tile([128, 4, 4 * PW], mbf, tag="vaugb")
            vaugc = big.tile([128, 4, 4 * PW], mbf, tag="vaugc")
            ctx4 = [big.tile([128, 4, 512], mbf, tag=f"ctx{c}",
                             name=f"ctx{c}")
                    for c in range(4)]
            sS = big.tile([64, 4, 64], mbf, tag="sS")        # softmax denoms
            rU = big.tile([64, 4, 64], mbf, tag="rU")

            def qk_ap(f, qc, rows=slice(None)):
                """qkT slice for feature chunk f, 512-token chunk qc."""
                if qc < 2:
                    return qkTa[rows, f, qc * 512:(qc + 1) * 512]
                return (qkTb if qc == 2 else qkTc)[rows, f, :]

            def k_ap(rows, f, kt):
                """k^T slice [64, 128] for one 128-token k-tile."""
                if kt < 8:
                    return qkTa[rows, f, kt * 128:(kt + 1) * 128]
                if kt < 12:
                    return qkTb[rows, f, (kt - 8) * 128:(kt - 7) * 128]
                return qkTc[rows, f, (kt - 12) * 128:(kt - 11) * 128]

            def v_tile(kt):
                if kt < 8:
                    return vauga, kt
                if kt < 12:
                    return vaugb, kt - 8
                return vaugc, kt - 12

            # HAM warm-up: keep the PE busy through the initial input-DMA
            # wait so the first real matmuls run at 2.4 GHz.
            warm = cpool.tile([128, 128], mbf, tag="warm")
            nc.vector.memset(warm, 0.0)
            wps = ps_sc.tile([128, 128], mf32, tag="sc", name="warmps")
            for i in range(60):
                nc.tensor.matmul(wps, warm, warm, start=(i == 0),
                                 stop=(i == 59))
            # load the exp table while the PE warms (first ACTIVATE pays
            # ~2.7us of table DMA otherwise)
            wexp = cpool.tile([128, 128], mbf, tag="wexp")
            nc.scalar.activation(wexp, wps, ACT.Exp, scale=0.125)

            # shared ones block of vaug: per pair [v_h0 | ones | v_h1],
            # so h0's stationary [v|ones] puts ctx on psum rows 0:64 and
            # h1's [ones|v] puts ctx on rows 64:128 -- both lane-aligned
            # with their ctx destination (no cross-partition bounce), and
            # rows 64:128 / 0:64 carry the softmax denominator replicas.
            for va in (vauga, vaugb, vaugc):
                vv = va.rearrange("p t (pr w) -> p t pr w", w=PW)
                nc.vector.memset(vv[:, :, :, 64:128], 1.0)

            def qk_evict(dst, acc, f):
                if with_bias:
                    nc.vector.tensor_scalar_add(dst, acc, bias[:, f:f + 1])
                else:
                    nc.vector.tensor_copy(dst, acc)

            def gen_qk_pair(fa, fb, qc):
                """One 512-col window of qkT for two f-chunks, matmuls
                interleaved (alternating psum banks), evictions on DVE."""
                acca = ps_a.tile([128, 512], mf32, tag="qkvp",
                                 name=f"qkq_{fa}_{qc}")
                accb = ps_a.tile([128, 512], mf32, tag="qkvp",
                                 name=f"qkq_{fb}_{qc}")
                for e in range(8):
                    nc.tensor.matmul(acca, wqk[:, fa, e, :],
                                     xT[:, qc, e, :],
                                     start=(e == 0), stop=(e == 7))
                    yield
                    nc.tensor.matmul(accb, wqk[:, fb, e, :],
                                     xT[:, qc, e, :],
                                     start=(e == 0), stop=(e == 7))
                    yield
                qk_evict(qk_ap(fa, qc), acca, fa)
                qk_evict(qk_ap(fb, qc), accb, fb)

            def vnat_evict(t, acc):
                va, ti = v_tile(t)
                vv = va.rearrange("p t (pr w) -> p t pr w", w=PW)
                src = acc.rearrange("p (pr j d) -> p pr j d", pr=4, j=2)
                bb = (bvb.rearrange("p (pr j d) -> p pr j d", pr=4, j=2)
                      if with_bias else None)
                for j, cols in ((0, slice(0, 64)), (1, slice(128, 192))):
                    if with_bias:
                        nc.vector.tensor_add(
                            vv[:, ti, :, cols], src[:, :, j, :], bb[:, :, j, :])
                    else:
                        nc.vector.tensor_copy(
                            vv[:, ti, :, cols], src[:, :, j, :])

            def gen_vnat_pair(ta, tb):
                """v rows (tokens) for two 128-token tiles, natural layout,
                x-stationary: v[t,:] = x[t,:] @ w_v."""
                acca = ps_a.tile([128, 512], mf32, tag="qkvp",
                                 name=f"vna_{ta}")
                accb = ps_a.tile([128, 512], mf32, tag="qkvp",
                                 name=f"vnb_{tb}")
                for e in range(8):
                    nc.tensor.matmul(
                        acca, xT[:, ta // 4, e, (ta % 4) * 128:(ta % 4 + 1) * 128],
                        wvT[:, e, :], start=(e == 0), stop=(e == 7))
                    yield
                    nc.tensor.matmul(
                        accb, xT[:, tb // 4, e, (tb % 4) * 128:(tb % 4 + 1) * 128],
                        wvT[:, e, :], start=(e == 0), stop=(e == 7))
                    yield
                vnat_evict(ta, acca)
                vnat_evict(tb, accb)

            def gen_cproj_t(t):
                """out[t-block] = ctx @ wp (row-parallel slice, f32), both
                512-col halves interleaved."""
                ct = ctx4[t // 4]
                tt = t % 4
                osb = outpool.tile([128, 1024], mbf, tag="osb",
                                   name=f"osb_{t}")
                pa = ps_a.tile([128, 512], mf32, tag="qkvp", name=f"cpa_{t}")
                pb = ps_a.tile([128, 512], mf32, tag="qkvp", name=f"cpb_{t}")
                for fc in range(4):
                    nc.tensor.matmul(pa, ct[:, fc, tt * 128:(tt + 1) * 128],
                                     wp[:, fc, 0:512],
                                     start=(fc == 0), stop=(fc == 3))
                    yield
                    nc.tensor.matmul(pb, ct[:, fc, tt * 128:(tt + 1) * 128],
                                     wp[:, fc, 512:1024],
                                     start=(fc == 0), stop=(fc == 3))
                    yield
                nc.vector.tensor_copy(osb[:, 0:512], pa)
                nc.vector.tensor_copy(osb[:, 512:1024], pb)
                nc.sync.dma_start(out=out_d[t * 128:(t + 1) * 128, :], in_=osb)

            class FillQueue:
                def __init__(self):
                    self.gens = []
                    self.cur = None

                def add(self, g):
                    self.gens.append(g)

                def pull(self, n):
                    for _ in range(n):
                        while True:
                            if self.cur is None:
                                if not self.gens:
                                    return
                                self.cur = self.gens.pop(0)
                            try:
                                next(self.cur)
                                break
                            except StopIteration:
                                self.cur = None

                def drain(self):
                    self.pull(1 << 30)

            fq = FillQueue()

            def attention_chunk(g2, c, nfill):
                nkt = 4 * c + 4
                ctxp = [ps_ctx.tile([128, 512], mf32, tag="ctx",
                                    name=f"ctxp{g2}_{c}_{jj}")
                        for jj in range(2)]

                def emit_ctx(kt, pv, off):
                    va, ti = v_tile(kt)
                    for j in range(2):
                        base = g2 * PW + 64 * j
                        nc.tensor.matmul(
                            ctxp[j][:, off:],
                            va[:, ti, base:base + 128],
                            pv[:, j, off:],
                            start=(kt == 0), stop=(kt == nkt - 1))

                pending_ctx = None
                for kt in range(nkt):
                    # Both heads' score matmuls row-tiled (concurrent in the
                    # PE array); halves of one [128,1024] psum tile ->
                    # single merged exp.  Diagonal k-tiles (m>=0) use exact
                    # column ranges.  The A.V matmul for kt is emitted after
                    # the scores of kt+1 so the exp it consumes has a k-tile
                    # of pipeline slack; fill matmuls sit before the A.V so
                    # the PE works while ACT does exp.
                    m = kt - 4 * c
                    off = 128 * m if m > 0 else 0
                    sc = ps_sc.tile([128, 1024], mf32, tag="sc",
                                    name=f"sc_{g2}_{c}_{kt}")
                    scv = sc.rearrange("r (j q) -> r j q", j=2)
                    for j in range(2):
                        rows = slice(64 * j, 64 * (j + 1))
                        nc.tensor.matmul(
                            scv[:, j, off:],
                            k_ap(rows, 4 + g2, kt),
                            qk_ap(g2, c, rows)[:, off:],
                            start=True, stop=True,
                            tile_position=(64 * j, 0))
                    p = ppool.tile([128, 1024], mbf, tag="p")
                    pv = p.rearrange("r (j q) -> r j q", j=2)
                    nc.scalar.activation(pv[:, :, off:], scv[:, :, off:],
                                         ACT.Exp, scale=0.125)
                    if m >= 0:
                        # lower-tri mask on the 128-wide diagonal block
                        nc.vector.tensor_mul(
                            pv[:, :, off:off + 128],
                            pv[:, :, off:off + 128],
                            cmask.rearrange("r (j q) -> r j q", j=2))
                    fq.pull(nfill)
                    if pending_ctx is not None:
                        emit_ctx(*pending_ctx)
                    pending_ctx = (kt, pv, off)
                emit_ctx(*pending_ctx)
                # the AV output is lane-aligned with its ctx destination
                # (j=0 ctx on psum rows 0:64, j=1 on 64:128); rows 64:128 /
                # 0:64 hold the denominator replicas.  Denominator rows go
                # to DRAM scratch reshaped [8 rows, 64] so the reciprocal
                # runs on 64 lanes with a 64-wide free dim.
                for j in range(2):
                    h = 2 * g2 + j
                    st = stpool.tile([65, 512], mbf, tag="st65",
                                     name=f"st_{g2}_{c}_{j}")
                    srow = 64 if j == 0 else 0
                    nc.vector.tensor_copy(st[srow:srow + 1, :],
                                          ctxp[j][srow:srow + 1, :])
                    nc.sync.dma_start(
                        out=sD_d[8 * h:8 * h + 8, 64 * c:64 * (c + 1)],
                        in_=st[srow:srow + 1, :])
                    nc.vector.tensor_copy(
                        ctx4[c][64 * j:64 * (j + 1), g2, :],
                        ctxp[j][64 * j:64 * (j + 1), :])

            # --- softmax-normalization chain, split so no engine FIFO ever
            # waits on a DMA round-trip: gather (DMA) emitted right after a
            # chunk; reciprocal + broadcast emitted ~a chunk later. ---
            def norm_gather(c, g2s=(0, 4)):
                lo, hi = 16 * g2s[0], 16 * g2s[1]
                nc.sync.dma_start(out=sS[lo:hi, c, :],
                                  in_=sD_d[lo:hi, 64 * c:64 * (c + 1)])

            def norm_finish(c, g2s=(0, 1, 2, 3), recip=True):
                ng = len(g2s)
                if recip:
                    lo, hi = 16 * g2s[0], 16 * (g2s[-1] + 1)
                    with nc.allow_low_precision(reason="1/s bf16 is plenty"):
                        nc.vector.reciprocal(rU[lo:hi, c, :], sS[lo:hi, c, :])
                    r0 = 8 * c + 2 * g2s[0]
                    r1 = 8 * c + 2 * g2s[-1] + 2
                    nc.sync.dma_start(
                        out=rU_d[r0:r1, :].rearrange(
                            "h (r q) -> (h r) q", r=8),
                        in_=rU[lo:hi, c, :])
                rb = rbpool.tile([128, 4, 512], mbf, tag="rb",
                                 name=f"rb_{c}_{g2s[0]}")
                for j in range(2):
                    base = rU_d[8 * c + 2 * g2s[0] + j:
                                8 * c + 2 * g2s[0] + j + 1, :]
                    bcast = bass.AP(tensor=base.tensor, offset=base.offset,
                                    ap=[[0, 64], [1024, ng], [1, 512]])
                    nc.sync.dma_start(out=rb[64 * j:64 * (j + 1), 0:ng, :],
                                      in_=bcast)
                return [(rb, i) for i in range(ng)]

            def norm_mul(c, rbs, g2s=(0, 1, 2, 3)):
                """ctx[c] *= 1/s (in place, bf16 2x mode)."""
                for (rb, i), g2 in zip(rbs, g2s):
                    for j in range(2):
                        sl = ctx4[c][64 * j:64 * (j + 1), g2, :]
                        nc.vector.tensor_mul(
                            sl, sl, rb[64 * j:64 * (j + 1), i, :])

            # ---- emission order = per-engine execution order ----
            # P1: q,k windows + v tiles, attention c0 staggered one window
            # pair behind its dependencies.
            for g in [gen_qk_pair(0, 4, 0), gen_qk_pair(0, 4, 1),
                      gen_vnat_pair(0, 1), gen_vnat_pair(2, 3),
                      gen_qk_pair(1, 5, 0), gen_qk_pair(1, 5, 1)]:
                for _ in g:
                    pass
            attention_chunk(0, 0, 0)
            for g in [gen_qk_pair(2, 6, 0), gen_qk_pair(2, 6, 1)]:
                for _ in g:
                    pass
            attention_chunk(1, 0, 0)
            for g in [gen_qk_pair(3, 7, 0), gen_qk_pair(3, 7, 1)]:
                for _ in g:
                    pass
            fq.add(gen_vnat_pair(4, 5))
            attention_chunk(2, 0, 2)
            fq.add(gen_vnat_pair(6, 7))
            attention_chunk(3, 0, 3)
            fq.drain()
            norm_gather(0)

            # c1: fill = qc=2 q,k windows; finish(0) after first chunk
            for fa, fb in [(0, 4), (1, 5), (2, 6), (3, 7)]:
                fq.add(gen_qk_pair(fa, fb, 2))
            attention_chunk(0, 1, 2)
            rbs0 = norm_finish(0)
            for g2 in range(1, 4):
                attention_chunk(g2, 1, 2)
            fq.drain()              # qc=2 must be done before c2 scores
            norm_gather(1)

            # c2: v tiles 8-11 precede (AV deps); fill = three qc=3
            # windows, v 12-15, then c0's first c_proj tiles
            for g in [gen_vnat_pair(8, 9), gen_vnat_pair(10, 11)]:
                for _ in g:
                    pass
            for fa, fb in [(0, 4), (1, 5), (2, 6)]:
                fq.add(gen_qk_pair(fa, fb, 3))
            fq.add(gen_vnat_pair(12, 13))
            fq.add(gen_vnat_pair(14, 15))
            attention_chunk(0, 2, 2)
            rbs1 = norm_finish(1)
            norm_mul(0, rbs0)
            for t in (0, 1):
                fq.add(gen_cproj_t(t))
            for g2 in range(1, 4):
                attention_chunk(g2, 2, 2)
            fq.drain()              # qc=3 (0-2) + v 12-15 before c3
            norm_gather(2)

            # c3: fill = last qc=3 window + c0/c1 c_proj; c2's c_proj is
            # held back to cover the tail.  The c3 normalization chain is
            # staggered per head-pair, one chunk behind its denominators.
            norm_mul(1, rbs1)
            fq.add(gen_qk_pair(3, 7, 3))
            for t in (2, 3):
                fq.add(gen_cproj_t(t))
            for t in range(4, 8):
                fq.add(gen_cproj_t(t))
            attention_chunk(0, 3, 2)
            rbs2 = norm_finish(2)
            norm_gather(3, (0, 1))
            attention_chunk(1, 3, 2)
            norm_mul(2, rbs2)
            norm_gather(3, (1, 2))
            attention_chunk(2, 3, 2)
            rb3 = norm_finish(3, (0, 1))
            norm_gather(3, (2, 3))
            attention_chunk(3, 3, 2)
            fq.drain()
            norm_gather(3, (3, 4))
            # tail: held-back c_proj (reads ctx[2], independent of the c3
            # norm chain) covers the g2=3 normalization round-trips; paced
            # dummy matmuls keyed on the chain keep HAM from re-throttling
            for t in (8, 9, 10, 11):
                fq.add(gen_cproj_t(t))
            fq.pull(32)
            wd1 = ps_sc.tile([128, 64], mf32, tag="sc", name="wd1")
            nc.tensor.matmul(wd1, warm[0:64, :], sS[:, 3, :],
                             start=True, stop=True)
            rb3 += norm_finish(3, (2, 3))
            fq.drain()
            wd2 = ps_sc.tile([128, 512], mf32, tag="sc", name="wd2")
            nc.tensor.matmul(wd2, warm, rb3[2][0][:, 0, :],
                             start=True, stop=True)
            norm_mul(3, rb3[0:1], (0,))
            norm_mul(3, rb3[1:2], (1,))
            norm_mul(3, rb3[2:3], (2,))
            norm_mul(3, rb3[3:4], (3,))
            for t in range(12, 16):
                for _ in gen_cproj_t(t):
                    pass

    nc.compile()
    return nc


def _prep_inputs(x, w_attn, b_attn, w_proj):
    """Host-side shard/layout prep for the 8 cores."""
    # causal masks: cmask[:, j*128 + q] = 1.0 iff q >= k_row
    k_r = np.arange(128)[:, None]
    q_i = np.arange(128)[None, :]
    tri = (q_i >= k_r)
    cmask = np.concatenate([tri, tri], axis=1).astype(BF16)  # [128, 256]

    # xT[qc, p, e*512+t'] = x[qc*512+t', e*128+p]: per-partition rows
    # are contiguous so input DMAs run at full bandwidth
    xT_b = [np.ascontiguousarray(
        x[b].T.reshape(8, 128, 4, 512).transpose(2, 1, 0, 3).reshape(
            4, 128, 4096)).astype(BF16) for b in range(B)]
    in_maps = []
    for core in range(NC_):
        b, g = core // 2, core % 2
        fsl = slice(g * GF, (g + 1) * GF)
        # q,k: [8, 128, 1024]; wqk[f, p, e*128+c] = w[e*128+p, base_f+c]
        wq = w_attn[:, fsl].reshape(8, 128, 4, 128)
        wk = w_attn[:, C + g * GF:C + (g + 1) * GF].reshape(8, 128, 4, 128)
        wqk = np.concatenate(
            [wq.transpose(2, 1, 0, 3).reshape(4, 128, 1024),
             wk.transpose(2, 1, 0, 3).reshape(4, 128, 1024)],
            axis=0).astype(BF16)
        # v: [128, 8, 512]; wvT[p, e, c] = w[e*128+p, 2C+g*512+c]
        wv = w_attn[:, 2 * C + g * GF:2 * C + (g + 1) * GF]
        wvT = np.ascontiguousarray(
            wv.reshape(8, 128, 512).transpose(1, 0, 2)).astype(BF16)
        bq = b_attn[fsl]
        bk = b_attn[C + g * GF:C + (g + 1) * GF]
        bias = np.stack(
            [np.concatenate([bq, bk])[f * 128:(f + 1) * 128]
             for f in range(8)], axis=1).astype(np.float32)
        bv = b_attn[2 * C + g * GF:2 * C + (g + 1) * GF].reshape(1, 512)
        bv = np.ascontiguousarray(bv).astype(np.float32)
        # wp[p, e*1024+t] = w_proj[g*512+e*128+p, t] (contiguous rows)
        wp = np.ascontiguousarray(
            w_proj[fsl, :].reshape(4, 128, 1024).transpose(1, 0, 2).reshape(
                128, 4096)).astype(BF16)
        in_maps.append({"xT": xT_b[b], "wqk": wqk, "wvT": wvT, "bias": bias,
                        "bv": bv, "wp": wp, "cmask": cmask})
    return in_maps


def _run(in_maps, trace=False, with_bias=False):
    from concourse.bass_utils import run_bass_kernel_spmd
    if with_bias not in _nc_cache:
        _nc_cache[with_bias] = _build(with_bias)
    return run_bass_kernel_spmd(_nc_cache[with_bias], in_maps,
                                core_ids=list(range(NC_)), trace=trace)


def kernel(x, w_attn, b_attn, w_proj, b_proj):
    x = np.asarray(x, dtype=np.float32)
    w_attn = np.asarray(w_attn, dtype=np.float32)
    b_attn = np.asarray(b_attn, dtype=np.float32)
    w_proj = np.asarray(w_proj, dtype=np.float32)
    b_proj = np.asarray(b_proj, dtype=np.float32)
    res = _run(_prep_inputs(x, w_attn, b_attn, w_proj),
               with_bias=bool(np.any(b_attn)))
    out = np.empty((B, T, C), np.float32)
    for b in range(B):
        out[b] = (res.results[2 * b]["out"].astype(np.float32)
                  + res.results[2 * b + 1]["out"].astype(np.float32)
                  + b_proj)
    return out
